# revision 35
# baseline (speedup 1.0000x reference)
"""GQA causal-attention prefill kernel for 8 TRN2 NeuronCores.

Sharding: tensor-parallel over heads. Core c owns q-heads {2c, 2c+1} and
kv-head c (whole GQA group). Each core computes its heads' attention and a
partial output projection; the host sums the 8 partials (no collectives).

Layout: activations kept transposed [feature, token] on-chip. Matmuls in
bf16 (PSUM f32). Softmax skips max-subtraction (scores bounded ~12).

v2 notes:
- rotate_half runs on DVE stream_shuffle: the head dim is permuted host-side
  so rope partners (d, d+64) sit 16 apart inside one 32-partition group
  (shuffles only permute within 32-partition groups); the sign of the
  rotation is folded into the sin table. QK dot products are invariant.
- softmax denominator: e-tiles are pair-summed on DVE (bf16) and reduced
  with one ones-matmul per quad of key blocks (PE work 29us -> 8.5us).
- exp is issued once per PAIR of key blocks over a [128,1024] psum tile
  spanning two banks (halves Scalar instruction+semaphore overhead).
- schedule: P0 | A0 interleaved with P1 fillers | A1 interleaved with W0
  fillers | W1, where fillers are pulled between attention pair-rounds so
  the in-order PE never waits on Scalar exp.
"""
import os
import sys

for _p in ("/opt/trn_rl_repo", "/root/.axon_site/_ro/trn_rl_repo"):
    if os.path.isdir(_p) and _p not in sys.path:
        sys.path.append(_p)

import numpy as np
import ml_dtypes
import concourse.bacc as bacc
import concourse.mybir as mybir
import concourse.tile as tile
from concourse.bass_utils import run_bass_kernel_spmd

B, S, DIM = 2, 2048, 2048
H, KVH, D = 16, 8, 128
NCORES = 8
HL = H // NCORES            # q heads per core
SQC = 512                   # sequence chunk (matmul moving dim)
NJ = S // SQC               # chunks per batch
KT = DIM // 128             # contraction tiles for the projections
BS = B * S
SCALE = 1.0 / float(np.sqrt(D))
SHUF16 = [(i + 16) % 32 for i in range(32)]   # swap 16-halves in each 32-group

F32 = mybir.dt.float32
BF16 = mybir.dt.bfloat16
AF = mybir.ActivationFunctionType


def build():
    nc = bacc.Bacc("TRN2", target_bir_lowering=False, debug=False,
                   num_devices=NCORES)
    xt = nc.dram_tensor("xt", [DIM, BS], BF16, kind="ExternalInput").ap()
    wq = nc.dram_tensor("wq", [128, KT * HL * D], BF16, kind="ExternalInput").ap()
    wk = nc.dram_tensor("wk", [128, KT * D], BF16, kind="ExternalInput").ap()
    wv = nc.dram_tensor("wv", [128, KT * D], BF16, kind="ExternalInput").ap()
    wo = nc.dram_tensor("wo", [HL * D, DIM], BF16, kind="ExternalInput").ap()
    gq = nc.dram_tensor("gq", [D, 1], F32, kind="ExternalInput").ap()
    gk = nc.dram_tensor("gk", [D, 1], F32, kind="ExternalInput").ap()
    cost = nc.dram_tensor("cost", [128, S], BF16, kind="ExternalInput").ap()
    sint = nc.dram_tensor("sint", [128, S], BF16, kind="ExternalInput").ap()
    mzt = nc.dram_tensor("mzt", [128, 512], BF16, kind="ExternalInput").ap()
    on128 = nc.dram_tensor("on128", [128, 128], BF16, kind="ExternalInput").ap()
    idn = nc.dram_tensor("idn", [128, 128], BF16, kind="ExternalInput").ap()
    out = nc.dram_tensor("out", [DIM, BS], BF16, kind="ExternalOutput").ap()

    with tile.TileContext(nc) as tc:
        with tc.tile_pool(name="const", bufs=1) as cp, \
             tc.tile_pool(name="xp", bufs=36) as xp, \
             tc.tile_pool(name="persist", bufs=2) as pp, \
             tc.tile_pool(name="wrk", bufs=2) as wrk, \
             tc.tile_pool(name="ep", bufs=6) as epool, \
             tc.tile_pool(name="dwk", bufs=3) as dwk, \
             tc.tile_pool(name="oop", bufs=8) as oop, \
             tc.tile_pool(name="stash", bufs=12) as stp, \
             tc.tile_pool(name="ps_sc", bufs=2, space="PSUM") as pssc, \
             tc.tile_pool(name="ps_pj", bufs=2, space="PSUM") as pspj, \
             tc.tile_pool(name="ps_av", bufs=1, space="PSUM") as psav, \
             tc.tile_pool(name="ps_row", bufs=1, space="PSUM") as psrow:

            # ---- constants / weights in SBUF ----
            # sync queue carries the startup-critical weights in strips so
            # consumers can start as soon as their strip lands; chunk-0 x
            # tiles go on the scalar/gpsimd queues in parallel.
            wk_sb = cp.tile([128, KT * D], BF16)
            wq_sb = cp.tile([128, KT * HL * D], BF16)
            wv_sb = cp.tile([128, KT * D], BF16)
            on128_sb = cp.tile([128, 128], BF16)
            gq_sb = cp.tile([D, 1], F32)
            gk_sb = cp.tile([D, 1], F32)
            idn_sb = cp.tile([128, 128], BF16)
            for st in range(4):
                w4 = 4 * D
                nc.sync.dma_start(out=wk_sb[:, st * w4:(st + 1) * w4],
                                  in_=wk[:, st * w4:(st + 1) * w4])
            wq4 = KT * HL * D // 4
            nc.sync.dma_start(out=wq_sb[:, 0:wq4], in_=wq[:, 0:wq4])
            nc.sync.dma_start(out=on128_sb[:], in_=on128)
            nc.sync.dma_start(out=gq_sb[:], in_=gq)
            nc.sync.dma_start(out=gk_sb[:], in_=gk)
            for st in range(1, 4):
                nc.sync.dma_start(out=wq_sb[:, st * wq4:(st + 1) * wq4],
                                  in_=wq[:, st * wq4:(st + 1) * wq4])
            nc.sync.dma_start(out=wv_sb[:], in_=wv)
            nc.sync.dma_start(out=idn_sb[:], in_=idn)
            # gpsimd: chunk-0 cos/sin columns ahead of the chunk-0 x odds
            cos_sb = cp.tile([128, S], BF16)
            sin_sb = cp.tile([128, S], BF16)
            nc.gpsimd.dma_start(out=cos_sb[:, 0:SQC], in_=cost[:, 0:SQC])
            nc.gpsimd.dma_start(out=sin_sb[:, 0:SQC], in_=sint[:, 0:SQC])
            mzt_sb = cp.tile([128, 512], BF16)
            wo_sb = [cp.tile([128, DIM], BF16, name=f"wo{h}") for h in range(HL)]

            def load_late_consts():
                nc.gpsimd.dma_start(out=cos_sb[:, SQC:], in_=cost[:, SQC:])
                nc.gpsimd.dma_start(out=sin_sb[:, SQC:], in_=sint[:, SQC:])
                yield
                nc.gpsimd.dma_start(out=mzt_sb[:], in_=mzt)
                yield

            # zero the score psum banks once: stale regions are exp'd before
            # being overwritten (then masked), so they must be finite.
            z0 = pssc.tile([128, 2 * SQC], F32, tag="sc")
            nc.vector.memset(z0[:], 0.0)
            z1 = pssc.tile([128, 2 * SQC], F32, tag="sc")
            nc.vector.memset(z1[:], 0.0)
            for _ in range(6):
                ez = epool.tile([128, 2 * SQC], BF16, tag="e", name="ez")
                nc.vector.memset(ez[:], 0.0)

            persist = {}
            for b in range(B):
                persist[b] = {
                    "qt": [pp.tile([128, S], BF16, tag=f"qt{h}", name=f"qt{h}_{b}")
                           for h in range(HL)],
                    "kt": pp.tile([128, S], BF16, tag="ktb", name=f"ktb_{b}"),
                    "v": pp.tile([128, S], BF16, tag="vb", name=f"vb_{b}"),
                    "ao": [pp.tile([128, S], BF16, tag=f"ao{h}", name=f"ao{h}_{b}")
                           for h in range(HL)],
                }

            def norm_pre(ps, g_col, defer=False):
                """emit sqr+tsb right after the matmuls: frees the proj psum
                bank as early as possible"""
                if defer:
                    sqr = stp.tile([128, SQC], BF16, tag="ssq", bufs=12)
                else:
                    sqr = wrk.tile([128, SQC], BF16, tag="sqr")
                nc.scalar.activation(sqr[:], ps[:], AF.Square)
                tsb = wrk.tile([128, SQC], BF16, tag="tsb")
                nc.scalar.activation(tsb[:], ps[:], AF.Copy, scale=g_col[:])
                return sqr, tsb

            def rope_u(sqr, tsb, dest, col, stash=None):
                """rope combine (DVE only, no tables). With stash, the
                rsqrt+apply is deferred to a later sweep so the Scalar
                stream never switches activation tables mid-attention."""
                rot = wrk.tile([128, SQC], BF16, tag="rot")
                nc.vector.stream_shuffle(rot[:], tsb[:], SHUF16)
                m1 = wrk.tile([128, SQC], BF16, tag="m1")
                nc.vector.tensor_mul(m1[:], tsb[:], cos_sb[:, col:col + SQC])
                m2 = wrk.tile([128, SQC], BF16, tag="m2")
                nc.vector.tensor_mul(m2[:], rot[:], sin_sb[:, col:col + SQC])
                if stash is None:
                    u = wrk.tile([128, SQC], BF16, tag="m1")
                else:
                    u = stp.tile([128, SQC], BF16, tag="su", bufs=12)
                nc.vector.tensor_add(u[:], m1[:], m2[:])
                return u

            def norm_apply(sqr, u, dest, col, pool=None, tag="row"):
                """rmsnorm scale: ssum matmul + rsqrt + apply"""
                ssum = (pool or psrow).tile([128, SQC], F32, tag=tag)
                nc.tensor.matmul(ssum[:], on128_sb[:], sqr[:],
                                 start=True, stop=True)
                bcs = wrk.tile([128, SQC], F32, tag="bcs")
                nc.scalar.activation(bcs[:], ssum[:], AF.Abs_reciprocal_sqrt,
                                     scale=1.0 / D)
                nc.vector.tensor_mul(dest[:, col:col + SQC], u[:], bcs[:])
                return bcs

            last_bcs_box = {}

            def norm_post(sqr, tsb, dest, col, stash=None):
                if stash is None:
                    u = rope_u(sqr, tsb, dest, col)
                    last_bcs_box["bcs"] = norm_apply(sqr, u, dest, col)
                else:
                    u = rope_u(sqr, tsb, dest, col, stash=True)
                    stash.append((sqr, u, dest, col))

            def proj_gen(b, stash=None):
                """Generator of filler quanta for batch b's projections.
                With stash, the rsqrt+apply stage is deferred (see
                norm_post)."""
                base = b * S
                pb = persist[b]
                for sc in range(NJ):
                    col = sc * SQC
                    first = b == 0 and sc == 0
                    xk = [xp.tile([128, SQC], BF16, tag="x", name=f"x{kt}")
                          for kt in range(KT)]

                    def emit_dma(kt, sc=sc, col=col):
                        if b == 0 and sc == 0:
                            # startup: sync is busy with weights
                            eng = (nc.scalar, nc.gpsimd)[kt % 2]
                        elif b == 0 and sc == 1:
                            # sync still streaming weights at this point
                            eng = (nc.gpsimd, nc.scalar)[kt % 2]
                        elif b == 0:
                            eng = (nc.sync, nc.gpsimd, nc.scalar)[kt % 3]
                        else:
                            eng = (nc.sync, nc.gpsimd)[kt % 2]
                        eng.dma_start(
                            out=xk[kt][:],
                            in_=xt[kt * 128:(kt + 1) * 128,
                                   base + col:base + col + SQC])

                    if not first:
                        for kt in range(KT):
                            emit_dma(kt)
                        yield
                    # order: k, q0, q1, v; norm for tensor t emitted during
                    # the matmuls of tensor t+1 so the PE never waits on the
                    # Scalar Square.
                    plan = [
                        ("k", wk_sb, 0, D, gk_sb, pb["kt"]),
                        ("q0", wq_sb, 0, HL * D, gq_sb, pb["qt"][0]),
                        ("q1", wq_sb, D, HL * D, gq_sb, pb["qt"][1]),
                    ]
                    prev = None
                    for nm, wsb, bcol, stride, g_col, dest in plan:
                        ps = pspj.tile([128, SQC], F32, tag="pj", name=f"pj_{nm}")
                        for kb in range(0, KT, 4):
                            for kt in range(kb, kb + 4):
                                if first and nm == "k":
                                    # chunk 0: emit each x dma right before
                                    # its matmul so the first matmuls wait
                                    # on exactly one tile each
                                    emit_dma(kt)
                                nc.tensor.matmul(
                                    ps[:],
                                    wsb[:, kt * stride + bcol:
                                        kt * stride + bcol + D],
                                    xk[kt][:], start=(kt == 0),
                                    stop=(kt == KT - 1))
                            yield
                        sqr, tsb = norm_pre(ps, g_col, defer=stash is not None)
                        if prev is not None:
                            norm_post(*prev, stash=stash)
                            yield
                        prev = (sqr, tsb, dest, col)
                    vp = pspj.tile([128, SQC], F32, tag="pj", name="pj_v")
                    for kb in range(0, KT, 4):
                        for kt in range(kb, kb + 4):
                            nc.tensor.matmul(
                                vp[:], wv_sb[:, kt * D:kt * D + D],
                                xk[kt][:], start=(kt == 0), stop=(kt == KT - 1))
                        yield
                    norm_post(*prev, stash=stash)  # q1
                    yield
                    # V: copy + transpose to natural [s, d] layout
                    vt = wrk.tile([128, SQC], BF16, tag="tsb")
                    nc.scalar.activation(vt[:], vp[:], AF.Copy)
                    vq = psrow.tile([128, SQC], BF16, tag="row")
                    for cq in range(4):
                        nc.tensor.transpose(vq[:, cq * 128:(cq + 1) * 128],
                                            vt[:, cq * 128:(cq + 1) * 128],
                                            idn_sb[:])
                    nc.vector.tensor_copy(pb["v"][:, col:col + SQC], vq[:])
                    yield

            def wo_gen(b, js, packed):
                """Filler quanta for the output projection of batch b,
                chunks in js. packed=True uses 2-bank sc tiles + paired
                casts (only when attention no longer needs the sc pool)."""
                base = b * S
                pb = persist[b]
                if not packed:
                    # spread as filler inside the A1 window: pj rotation,
                    # casts mostly on DVE (Scalar is exp-bound there)
                    for j in js:
                        for dt in range(KT):
                            op = pspj.tile([128, SQC], F32, tag="pj",
                                           name="pj_o")
                            for h in range(HL):
                                nc.tensor.matmul(
                                    op[:], wo_sb[h][:, dt * 128:(dt + 1) * 128],
                                    pb["ao"][h][:, j * SQC:(j + 1) * SQC],
                                    start=(h == 0), stop=(h == HL - 1))
                            oo = oop.tile([128, SQC], BF16, tag="oo")
                            if dt % 4 == 3:
                                nc.scalar.activation(oo[:], op[:], AF.Copy)
                            else:
                                nc.vector.tensor_copy(oo[:], op[:])
                            nc.sync.dma_start(
                                out=out[dt * 128:(dt + 1) * 128,
                                        base + j * SQC:base + (j + 1) * SQC],
                                in_=oo[:])
                            yield
                else:
                    for j in js:
                        for dp in range(0, KT, 2):
                            op = pssc.tile([128, 2 * SQC], F32, tag="sc",
                                           name="sc_o")
                            for half in (0, 1):
                                dt = dp + half
                                for h in range(HL):
                                    nc.tensor.matmul(
                                        op[:, half * SQC:(half + 1) * SQC],
                                        wo_sb[h][:, dt * 128:(dt + 1) * 128],
                                        pb["ao"][h][:, j * SQC:(j + 1) * SQC],
                                        start=(h == 0), stop=(h == HL - 1))
                            oo = oop.tile([128, 2 * SQC], BF16, tag="oo2",
                                          bufs=6)
                            if (dp // 2) % 2 == 0:
                                nc.scalar.activation(oo[:], op[:], AF.Copy)
                            else:
                                nc.vector.tensor_copy(oo[:], op[:])
                            for half in (0, 1):
                                dt = dp + half
                                (nc.sync, nc.gpsimd)[half].dma_start(
                                    out=out[dt * 128:(dt + 1) * 128,
                                            base + j * SQC:base + (j + 1) * SQC],
                                    in_=oo[:, half * SQC:(half + 1) * SQC])
                            yield

            class Filler:
                def __init__(self):
                    self.gens = []

                def add(self, g):
                    self.gens.append(g)

                def pull(self, n):
                    for _ in range(n):
                        while self.gens:
                            try:
                                next(self.gens[0])
                                break
                            except StopIteration:
                                self.gens.pop(0)
                        else:
                            return

                def drain(self):
                    while self.gens:
                        try:
                            next(self.gens[0])
                        except StopIteration:
                            self.gens.pop(0)

            def attn_chunk(b, h, j, filler, exp_bias=0.0):
                pb = persist[b]
                nsk = 4 * j + 4
                npair = nsk // 2
                avp = psav.tile([128, SQC], F32, tag="av")
                qt = pb["qt"][h]
                ktb = pb["kt"]
                vb = pb["v"]
                pend = None     # masks+AV+den of the previous pair (sw pipeline)
                state = {"tp": None, "acc": None}
                for m in range(npair):
                    sc_t = pssc.tile([128, 2 * SQC], F32, tag="sc")
                    for half in (0, 1):
                        i = 2 * m + half
                        lo = max(0, (i - 4 * j) * 128)
                        nc.tensor.matmul(
                            sc_t[:, half * SQC + lo:(half + 1) * SQC],
                            ktb[:, i * 128:(i + 1) * 128],
                            qt[:, j * SQC + lo:(j + 1) * SQC],
                            start=True, stop=True)
                    e = epool.tile([128, 2 * SQC], BF16, tag="e")
                    xlo = 256 if m == 2 * j + 1 else 0
                    nc.scalar.activation(e[:, xlo:], sc_t[:, xlo:], AF.Exp,
                                         scale=SCALE, bias=exp_bias)
                    if pend is not None:
                        pend()
                        filler.pull(1)
                    mm = m

                    def fin(e=e, mm=mm):
                        for half in (0, 1):
                            i = 2 * mm + half
                            c = i - 4 * j
                            if c >= 0:
                                w = (c + 1) * 128
                                nc.vector.tensor_mul(
                                    e[:, half * SQC:half * SQC + w],
                                    e[:, half * SQC:half * SQC + w],
                                    mzt_sb[:, SQC - w:SQC])
                        for half in (0, 1):
                            i = 2 * mm + half
                            lo = max(0, (i - 4 * j) * 128)
                            nc.tensor.matmul(
                                avp[:, lo:], vb[:, i * 128:(i + 1) * 128],
                                e[:, half * SQC + lo:(half + 1) * SQC],
                                start=(i == 0), stop=(i == nsk - 1))
                        # denominator partial sums (bf16, quad tree then chain)
                        t = dwk.tile([128, SQC], BF16, tag="dt")
                        nc.vector.tensor_add(t[:], e[:, 0:SQC],
                                             e[:, SQC:2 * SQC])
                        if mm % 2 == 0:
                            state["tp"] = t
                        else:
                            q = dwk.tile([128, SQC], BF16, tag="dq")
                            nc.vector.tensor_add(q[:], state["tp"][:], t[:])
                            if state["acc"] is None:
                                state["acc"] = q
                            else:
                                a2 = dwk.tile([128, SQC], BF16, tag="da")
                                nc.vector.tensor_add(a2[:], state["acc"][:], q[:])
                                state["acc"] = a2
                    pend = fin
                pend()
                # single short-lived den matmul -> reciprocal -> scale AV
                dnp = psrow.tile([128, SQC], F32, tag="row")
                nc.tensor.matmul(dnp[:], on128_sb[:], state["acc"][:],
                                 start=True, stop=True)
                rec = wrk.tile([128, SQC], F32, tag="rec")
                nc.vector.reciprocal_approx_fast(rec[:], dnp[:])
                nc.vector.tensor_mul(
                    pb["ao"][h][:, j * SQC:(j + 1) * SQC], avp[:], rec[:])
                filler.pull(2)

            # ================= schedule =================
            # P0, with the late consts streamed in between early quanta
            f0 = Filler()
            f0.add(proj_gen(0))
            lc = load_late_consts()
            f0.pull(8)          # chunk-0 k (x quarters) + q0
            next(lc)            # cos/sin rest (needed from chunk 1 on)
            f0.pull(6)          # norm_k + q1 + norm_q0
            next(lc)            # mzt, wo
            f0.drain()
            # gate A0's exps on P0's last rsqrt: the scheduler otherwise
            # interleaves Sqrt-table and Exp-table activations, paying a
            # 1.3us table reload each way.
            bias0 = cp.tile([128, 1], F32, name="bias0")
            nc.vector.tensor_scalar_mul(bias0[:], last_bcs_box["bcs"][:, :1],
                                        0.0)
            # A0 (+ P1 fillers; P1's rsqrt stage is stashed)
            # wo weights are first needed ~100us in; load them now so they
            # never displace x prefetch during P0
            for h in range(HL):
                nc.gpsimd.dma_start(out=wo_sb[h][:],
                                    in_=wo[h * 128:(h + 1) * 128, :])
            p1_stash = []
            f1 = Filler()
            f1.add(proj_gen(1, stash=p1_stash))
            for j in range(NJ):
                for h in range(HL):
                    attn_chunk(0, h, j, f1, exp_bias=bias0[:, :1])
                f1.pull(2)
            f1.drain()
            # P1 norm sweep: all 12 rsqrts back-to-back (one activation
            # table switch instead of one per chunk); ssum rows go through
            # the idle av bank so W0 fillers (pj banks) never interlock.
            f2 = Filler()
            f2.add(wo_gen(0, js=[0, 1, 2, 3], packed=False))
            pools = [(psav, "av"), (psrow, "row")]
            for k, ent in enumerate(p1_stash):
                swb = norm_apply(*ent, pool=pools[k % 2][0], tag=pools[k % 2][1])
                f2.pull(2)
            bias1 = cp.tile([128, 1], F32, name="bias1")
            nc.vector.tensor_scalar_mul(bias1[:], swb[:, :1], 0.0)
            # A1 (+ W0 fillers, then W1 chunks fed in as they complete)
            for j in range(NJ):
                for h in range(HL):
                    attn_chunk(1, h, j, f2, exp_bias=bias1[:, :1])
                if j < NJ - 1:
                    f2.add(wo_gen(1, js=[j], packed=False))
                f2.pull(2)
            f2.drain()
            # W tail: the last W1 chunk, pair-packed on the freed
            # attention psum banks
            f3 = Filler()
            f3.add(wo_gen(1, js=[NJ - 1], packed=True))
            f3.drain()
    nc.compile()
    return nc


_NC_CACHE = None


def _get_nc():
    global _NC_CACHE
    if _NC_CACHE is None:
        _NC_CACHE = build()
    return _NC_CACHE


def _bf(a):
    return np.ascontiguousarray(a.astype(ml_dtypes.bfloat16))


def _tile_w(w):
    cols = w.shape[1]
    return _bf(w.reshape(KT, 128, cols).transpose(1, 0, 2).reshape(128, KT * cols))


# head-dim permutation: rope partner (d, d+64) -> same 32-partition group,
# 16 apart, so DVE stream_shuffle can realize rotate_half.
_PERM = np.empty(128, np.int64)
_SIGN = np.empty(128, np.float32)
for _g in range(4):
    for _s in range(32):
        _p = _g * 32 + _s
        if _s < 16:
            _PERM[_p] = _g * 16 + _s
            _SIGN[_p] = -1.0
        else:
            _PERM[_p] = 64 + _g * 16 + (_s - 16)
            _SIGN[_p] = 1.0


def kernel(x, wq, wk, wv, wo, q_gamma, k_gamma, cos_cache, sin_cache):
    x = np.asarray(x, dtype=np.float32)
    wq = np.asarray(wq, dtype=np.float32)
    wk = np.asarray(wk, dtype=np.float32)
    wv = np.asarray(wv, dtype=np.float32)
    wo = np.asarray(wo, dtype=np.float32)
    q_gamma = np.asarray(q_gamma, dtype=np.float32)
    k_gamma = np.asarray(k_gamma, dtype=np.float32)
    cos_cache = np.asarray(cos_cache, dtype=np.float32)
    sin_cache = np.asarray(sin_cache, dtype=np.float32)

    xt = _bf(x.reshape(BS, DIM).T)
    # permuted rope tables; sin carries the rotation sign
    cos_t = cos_cache[:S].T           # [64, S]
    sin_t = sin_cache[:S].T
    cost = _bf(cos_t[_PERM % 64])
    sint = _bf(sin_t[_PERM % 64] * _SIGN[:, None])
    gq = np.ascontiguousarray(q_gamma[_PERM][:, None])
    gk = np.ascontiguousarray(k_gamma[_PERM][:, None])
    p = np.arange(128)[:, None]
    c = np.arange(512)[None, :]
    # [zeros(384) | lower-tri(128)]: slicing the last (c+1)*128 cols masks a
    # diagonal block at position c (zeros below the tri block).
    mzt = _bf(((c >= 384) & (p <= c - 384)).astype(np.float32))
    on128 = _bf(np.ones((128, 128), np.float32))
    idn = _bf(np.eye(128, dtype=np.float32))

    # permute head-dim columns of wq/wk
    wq_p = wq.reshape(DIM, H, D)[:, :, _PERM].reshape(DIM, H * D)
    wk_p = wk.reshape(DIM, KVH, D)[:, :, _PERM].reshape(DIM, KVH * D)

    in_maps = []
    for cid in range(NCORES):
        in_maps.append({
            "xt": xt,
            "wq": _tile_w(wq_p[:, cid * HL * D:(cid + 1) * HL * D]),
            "wk": _tile_w(wk_p[:, cid * D:(cid + 1) * D]),
            "wv": _tile_w(wv[:, cid * D:(cid + 1) * D]),
            "wo": _bf(wo[cid * HL * D:(cid + 1) * HL * D, :]),
            "gq": gq, "gk": gk, "cost": cost, "sint": sint,
            "mzt": mzt, "on128": on128, "idn": idn,
        })

    nc = _get_nc()
    trace = os.environ.get("KERNEL_TRACE") == "1"
    r = run_bass_kernel_spmd(nc, in_maps, core_ids=list(range(NCORES)),
                             trace=trace)
    if trace:
        kernel.last_exec_time_ns = r.exec_time_ns
        kernel.last_results = r
    acc = np.zeros((DIM, BS), np.float32)
    for cid in range(NCORES):
        acc += r.results[cid]["out"].astype(np.float32)
    return np.ascontiguousarray(
        acc.T.reshape(B, S, DIM).astype(np.float32))
